# revision 18
# baseline (speedup 1.0000x reference)
"""Trainium2 Bass kernel for nn_AOP_NN_49168785604711 (gnn_message_passing).

Two-layer ontology MLP:
  layer0: 8192 terms, each Linear(64->8)+tanh+BN(8), aux Linear(8->2)+tanh+BN(2)
  layer1: 1024 roots, concat 8 children (64) -> Linear(64->8)+tanh+BN, aux same.

Sharding: term axis split across 8 cores (expert parallel, no collectives).
Per core: 1024 MIE terms / 128 roots. x replicated.

Device layout: (term,feature) on SBUF partitions, batch on the free axis.
BatchNorm batch stats are then native free-axis reductions (bn_stats).
Outputs are written in [term*feat, B] layout; the host transposes (untimed).
"""

import os
import sys
import numpy as np

sys.path.insert(0, "/opt/trn_rl_repo")

from concourse import bass, bacc, mybir, tile  # noqa: E402

F32 = mybir.dt.float32
F32R = mybir.dt.float32r
BF16 = mybir.dt.bfloat16
I32 = mybir.dt.int32

B = 2048
D_IN = 64
H = 8
M0 = 8192
M1 = 1024
FANIN = 8
N_CORES = 8
EPS = 1e-5

M0L = M0 // N_CORES          # 1024 terms per core
M1L = M1 // N_CORES          # 128 roots per core
TG = 16                      # terms per group (16*8 = 128 partitions)
NG = M0L // TG               # 64 groups per core
NP = NG // 2                 # 32 group-pairs
NT1 = M1L * H // 128         # 8 t1 tiles of [128, B]
NC_FREE = 4                  # B split into 4 chunks of 512 for matmul/bn_stats
BC = B // NC_FREE            # 512

MAGIC = 0x5F3759DF

# matmul operand dtype: float32r streams 1 row/cycle (vs 4 for float32)
MM_DT = F32R if os.environ.get("KERNEL_MM_F32R", "1") == "1" else F32
# tanh-output dtype: bf16 halves bn_stats + apply input traffic
ACT_DT = BF16 if os.environ.get("KERNEL_BF16_ACT", "1") == "1" else F32

LAST_EXEC_TIME_NS = None


NR_ITERS = int(os.environ.get("KERNEL_NR_ITERS", "1"))


def _bn_scale_bias(nc, pool, mean, ve, gcols, becols, magic, p, n, tag):
    """BN scalars from mean [p,n] and ve=var+eps [p,n] -> s, c [p,n].
    rstd via quake seed + NR_ITERS Newton steps (all on DVE)."""
    y = pool.tile([p, n], F32, tag=f"nw_y{tag}", name=f"y_{tag}")
    t = pool.tile([p, n], F32, tag=f"nw_t{tag}", name=f"t_{tag}")
    sh = pool.tile([p, n], I32, tag=f"nw_s{tag}", name=f"sh_{tag}")
    nc.vector.tensor_scalar(sh[:], ve[:].bitcast(I32), 1, None,
                            mybir.AluOpType.logical_shift_right)
    nc.vector.tensor_tensor(y[:].bitcast(I32), magic[:p, :n], sh[:],
                            mybir.AluOpType.subtract)
    for _ in range(NR_ITERS):
        nc.vector.tensor_tensor(t[:], y[:], y[:], mybir.AluOpType.mult)
        nc.vector.tensor_tensor(t[:], t[:], ve[:], mybir.AluOpType.mult)
        nc.vector.tensor_scalar(t[:], t[:], -0.5, 1.5,
                                mybir.AluOpType.mult, mybir.AluOpType.add)
        nc.vector.tensor_tensor(y[:], y[:], t[:], mybir.AluOpType.mult)
    s = pool.tile([p, n], F32, tag=f"bn_s{tag}", name=f"s_{tag}")
    c = pool.tile([p, n], F32, tag=f"bn_c{tag}", name=f"c_{tag}")
    nc.vector.tensor_tensor(s[:], gcols, y[:], mybir.AluOpType.mult)
    nc.vector.tensor_tensor(c[:], mean[:], s[:], mybir.AluOpType.mult)
    nc.vector.tensor_tensor(c[:], becols, c[:], mybir.AluOpType.subtract)
    return s, c


def _moments(nc, small, big, src_tile, p, macc_sum_ap, msq_slot_ap, tag):
    """E[x] path: macc already holds per-instr tanh sums; caller passes the
    summed-over-chunks AP.  E[x^2]+EPS via one tensor_tensor_reduce."""
    sq = big.tile([p, NC_FREE, BC], ACT_DT, tag="sqscr", name=f"sq_{tag}",
                  bufs=2)
    nc.vector.scalar_tensor_tensor(
        out=sq[:], in0=src_tile[:], scalar=1.0, in1=src_tile[:],
        op0=mybir.AluOpType.mult, op1=mybir.AluOpType.mult,
        accum_out=msq_slot_ap)


def build(n_groups=NG):
    """Build the per-core Bass graph (SPMD: same graph, per-core weights)."""
    nc = bacc.Bacc(target_bir_lowering=False, debug=False)
    n_pairs = n_groups // 2
    n_t1 = max(1, n_groups // 8)

    # ---- dram parameters (per-core shards, host-prepped layouts) ----
    xT = nc.declare_dram_parameter("xT", [D_IN, B], MM_DT, isOutput=False)
    w0t = nc.declare_dram_parameter("w0t", [D_IN, n_groups * 128], MM_DT, isOutput=False)
    comb = nc.declare_dram_parameter("comb", [n_groups, 128, 96], MM_DT, isOutput=False)
    comb1 = nc.declare_dram_parameter("comb1", [128, n_t1, 32], MM_DT, isOutput=False)
    b0t = nc.declare_dram_parameter("b0t", [128, n_groups], F32, isOutput=False)
    g0t = nc.declare_dram_parameter("g0t", [128, n_groups], F32, isOutput=False)
    be0t = nc.declare_dram_parameter("be0t", [128, n_groups], F32, isOutput=False)
    bauxt = nc.declare_dram_parameter("bauxt", [96, n_pairs], F32, isOutput=False)
    gauxt = nc.declare_dram_parameter("gauxt", [96, n_pairs], F32, isOutput=False)
    beauxt = nc.declare_dram_parameter("beauxt", [96, n_pairs], F32, isOutput=False)
    b1at = nc.declare_dram_parameter("b1at", [32, n_t1], F32, isOutput=False)
    ga1t = nc.declare_dram_parameter("ga1t", [32, n_t1], F32, isOutput=False)
    bea1t = nc.declare_dram_parameter("bea1t", [32, n_t1], F32, isOutput=False)

    t0o = nc.declare_dram_parameter("t0o", [n_groups * 128, B], MM_DT, isOutput=True)
    a0o = nc.declare_dram_parameter("a0o", [n_groups * 32, B], MM_DT, isOutput=True)
    t1o = nc.declare_dram_parameter("t1o", [n_t1 * 128, B], MM_DT, isOutput=True)
    a1o = nc.declare_dram_parameter("a1o", [n_t1 * 32, B], F32, isOutput=True)

    with tile.TileContext(nc) as tc:
        with (
            tc.tile_pool(name="const", bufs=1) as cpool,
            tc.tile_pool(name="wload", bufs=3) as wpool,
            tc.tile_pool(name="big", bufs=3) as big,
            tc.tile_pool(name="auxbuf", bufs=3) as auxbuf,
            tc.tile_pool(name="t1pool", bufs=2) as t1pool,
            tc.tile_pool(name="small", bufs=6) as small,
            tc.tile_pool(name="pmain", bufs=2, space="PSUM") as pmain,
            tc.tile_pool(name="paux", bufs=4, space="PSUM") as paux,
        ):
            # ---- persistent constants ----
            xt_s = cpool.tile([D_IN, B], MM_DT)
            nc.sync.dma_start(xt_s[:], xT[:, :])
            b0_s = cpool.tile([128, n_groups], F32)
            g0_s = cpool.tile([128, n_groups], F32)
            be0_s = cpool.tile([128, n_groups], F32)
            nc.sync.dma_start(b0_s[:], b0t[:, :])
            nc.sync.dma_start(g0_s[:], g0t[:, :])
            nc.sync.dma_start(be0_s[:], be0t[:, :])
            baux_s = cpool.tile([96, n_pairs], F32)
            gaux_s = cpool.tile([96, n_pairs], F32)
            beaux_s = cpool.tile([96, n_pairs], F32)
            nc.sync.dma_start(baux_s[:], bauxt[:, :])
            nc.sync.dma_start(gaux_s[:], gauxt[:, :])
            nc.sync.dma_start(beaux_s[:], beauxt[:, :])
            c1_s = cpool.tile([128, n_t1, 32], MM_DT)
            nc.sync.dma_start(c1_s[:], comb1[:, :, :])
            b1a_s = cpool.tile([32, n_t1], F32)
            ga1_s = cpool.tile([32, n_t1], F32)
            bea1_s = cpool.tile([32, n_t1], F32)
            nc.sync.dma_start(b1a_s[:], b1at[:, :])
            nc.sync.dma_start(ga1_s[:], ga1t[:, :])
            nc.sync.dma_start(bea1_s[:], bea1t[:, :])
            magic = cpool.tile([128, 8], I32)
            nc.vector.memset(magic[:], MAGIC)

            t1buf = None
            auxt_ring = {}
            for p in range(n_pairs):
                if p % 4 == 0:
                    t1buf = t1pool.tile([128, NC_FREE, BC], MM_DT, tag="t1buf")
                    meanA4 = small.tile([96, 4], F32, tag="meanA4",
                                        name=f"meanA4_{p}")
                    msqA4 = small.tile([96, 4], F32, tag="msqA4",
                                       name=f"msqA4_{p}")
                psa = [paux.tile([96, BC], F32, tag="paux",
                                 name=f"psa_{p}_{i}") for i in range(NC_FREE)]
                h0s, cbs, t0ts = [], [], []
                macc = small.tile([128, 2, 2], F32, tag="macc",
                                  name=f"macc_{p}")
                msq2 = small.tile([128, 2], F32, tag="msq2",
                                  name=f"msq2_{p}")
                for q in (0, 1):
                    g = 2 * p + q
                    # ---- layer0 main: h0 = tanh(x @ W0 + b0) ----
                    w0g = wpool.tile([D_IN, 128], MM_DT, tag="w0g",
                                     name=f"w0g_{g}")
                    nc.sync.dma_start(w0g[:], w0t[:, g * 128:(g + 1) * 128])
                    cb = wpool.tile([128, 96], MM_DT, tag="cb", name=f"cb_{g}")
                    nc.sync.dma_start(cb[:], comb[g, :, :])
                    cbs.append(cb)
                    h0 = big.tile([128, NC_FREE, BC], ACT_DT, tag="h0",
                                  name=f"h0_{g}", bufs=4)
                    h0s.append(h0)
                    for half in (0, 1):
                        ps = pmain.tile([128, 2, BC], F32, tag="pmain",
                                        name=f"ps_{g}_{half}")
                        for j in (0, 1):
                            nc.tensor.matmul(
                                ps[:, j, :],
                                w0g[:],
                                xt_s[:, (2 * half + j) * BC:(2 * half + j + 1) * BC],
                                start=True, stop=True)
                        nc.scalar.activation(
                            h0[:, 2 * half:2 * half + 2, :], ps[:, :, :],
                            mybir.ActivationFunctionType.Tanh,
                            bias=b0_s[:, g:g + 1],
                            accum_out=macc[:, q, half:half + 1])
                    _moments(nc, small, big, h0, 128, None,
                             msq2[:, q:q + 1], f"m{g}")
                # ---- batched main-BN scalars for the pair ----
                mean2 = small.tile([128, 2], F32, tag="mean2",
                                   name=f"mean2_{p}")
                ve2 = small.tile([128, 2], F32, tag="ve2", name=f"ve2_{p}")
                nc.vector.tensor_tensor(
                    mean2[:], macc[:, :, 0:1].rearrange("p a b -> p (a b)"),
                    macc[:, :, 1:2].rearrange("p a b -> p (a b)"),
                    mybir.AluOpType.add)
                nc.vector.tensor_scalar(mean2[:], mean2[:], 1.0 / B, None,
                                        mybir.AluOpType.mult)
                nc.vector.tensor_scalar(msq2[:], msq2[:], 1.0 / B, EPS,
                                        mybir.AluOpType.mult,
                                        mybir.AluOpType.add)
                nc.vector.tensor_tensor(ve2[:], mean2[:], mean2[:],
                                        mybir.AluOpType.mult)
                nc.vector.tensor_tensor(ve2[:], msq2[:], ve2[:],
                                        mybir.AluOpType.subtract)
                s2, c2 = _bn_scale_bias(nc, small, mean2, ve2,
                                        g0_s[:, 2 * p:2 * p + 2],
                                        be0_s[:, 2 * p:2 * p + 2],
                                        magic, 128, 2, "m")
                for q in (0, 1):
                    g = 2 * p + q
                    t0t = big.tile([128, NC_FREE, BC], MM_DT, tag="t0t",
                                   name=f"t0t_{g}", bufs=4)
                    t0ts.append(t0t)
                    nc.gpsimd.tensor_scalar(t0t[:], h0s[q][:],
                                            s2[:, q:q + 1], c2[:, q:q + 1],
                                            mybir.AluOpType.mult,
                                            mybir.AluOpType.add)
                    nc.sync.dma_start(t0o[g * 128:(g + 1) * 128, :],
                                      t0t[:].rearrange("p a b -> p (a b)"))
                    # ---- aux0 + layer1 matmuls (block-diag accumulate) ----
                    for j in range(NC_FREE):
                        nc.tensor.matmul(
                            psa[j][:, :],
                            cbs[q][:],
                            t0t[:, j, :],
                            start=(q == 0), stop=(q == 1))
                # ---- pair epilogue: tanh into packed [96, B] tile ----
                auxt = auxbuf.tile([96, NC_FREE, BC], ACT_DT, tag="auxt",
                                   name=f"auxt_{p}", bufs=5)
                auxt_ring[p] = auxt
                acca = small.tile([96, NC_FREE], F32, tag="acca",
                                  name=f"acca_{p}")
                for j in range(NC_FREE):
                    nc.scalar.activation(auxt[:, j, :], psa[j][:, :],
                                         mybir.ActivationFunctionType.Tanh,
                                         bias=baux_s[:, p:p + 1],
                                         accum_out=acca[:, j:j + 1])
                nc.vector.tensor_reduce(meanA4[:, p % 4:p % 4 + 1], acca[:],
                                        mybir.AxisListType.X,
                                        mybir.AluOpType.add)
                _moments(nc, small, big, auxt, 96, None,
                         msqA4[:, p % 4:p % 4 + 1], f"a{p}")

                # ---- every 4 pairs: batched aux-BN scalars + epilogues ----
                if p % 4 == 3:
                    p0 = p - 3
                    veA4 = small.tile([96, 4], F32, tag="veA4",
                                      name=f"veA4_{p}")
                    nc.vector.tensor_scalar(meanA4[:], meanA4[:], 1.0 / B,
                                            None, mybir.AluOpType.mult)
                    nc.vector.tensor_scalar(msqA4[:], msqA4[:], 1.0 / B,
                                            EPS, mybir.AluOpType.mult,
                                            mybir.AluOpType.add)
                    nc.vector.tensor_tensor(veA4[:], meanA4[:], meanA4[:],
                                            mybir.AluOpType.mult)
                    nc.vector.tensor_tensor(veA4[:], msqA4[:], veA4[:],
                                            mybir.AluOpType.subtract)
                    sa4, ca4 = _bn_scale_bias(nc, small, meanA4, veA4,
                                              gaux_s[:, p0:p0 + 4],
                                              beaux_s[:, p0:p0 + 4],
                                              magic, 96, 4, "a")
                    for k in range(4):
                        pk = p0 + k
                        auxa = auxbuf.tile([96, NC_FREE, BC], MM_DT,
                                           tag="auxa", name=f"auxa_{pk}",
                                           bufs=2)
                        nc.gpsimd.tensor_scalar(auxa[:], auxt_ring.pop(pk)[:],
                                                sa4[:, k:k + 1],
                                                ca4[:, k:k + 1],
                                                mybir.AluOpType.mult,
                                                mybir.AluOpType.add)
                        ge, go = 2 * pk, 2 * pk + 1
                        nc.sync.dma_start(
                            a0o[ge * 32:(ge + 1) * 32, :],
                            auxa[0:32].rearrange("p a b -> p (a b)"))
                        nc.sync.dma_start(
                            a0o[go * 32:(go + 1) * 32, :],
                            auxa[48:80].rearrange("p a b -> p (a b)"))
                        sl_e = (ge % 8) * 16
                        sl_o = (go % 8) * 16
                        nc.sync.dma_start(t1buf[sl_e:sl_e + 16, :, :],
                                          auxa[32:48, :, :])
                        nc.sync.dma_start(t1buf[sl_o:sl_o + 16, :, :],
                                          auxa[80:96, :, :])

                    # ---- t1 tile complete -> write out + aux1 chain ----
                    t = p // 4
                    nc.sync.dma_start(t1o[t * 128:(t + 1) * 128, :],
                                      t1buf[:].rearrange("p a b -> p (a b)"))
                    a1t = auxbuf.tile([32, NC_FREE, BC], ACT_DT, tag="a1t",
                                      name=f"a1t_{t}", bufs=2)
                    acc1 = small.tile([32, NC_FREE], F32, tag="acc1",
                                      name=f"acc1_{t}")
                    for j in range(NC_FREE):
                        ps1 = paux.tile([32, BC], F32, tag="paux",
                                        name=f"ps1_{t}_{j}")
                        nc.tensor.matmul(ps1[:, :],
                                         c1_s[:, t, :],
                                         t1buf[:, j, :],
                                         start=True, stop=True)
                        nc.scalar.activation(a1t[:, j, :], ps1[:, :],
                                             mybir.ActivationFunctionType.Tanh,
                                             bias=b1a_s[:, t:t + 1],
                                             accum_out=acc1[:, j:j + 1])
                    mean1 = small.tile([32, 1], F32, tag="mean1",
                                       name=f"mean1_{t}")
                    msq1 = small.tile([32, 1], F32, tag="msq1",
                                      name=f"msq1_{t}")
                    ve1 = small.tile([32, 1], F32, tag="ve1",
                                     name=f"ve1_{t}")
                    nc.vector.tensor_reduce(mean1[:], acc1[:],
                                            mybir.AxisListType.X,
                                            mybir.AluOpType.add)
                    nc.vector.tensor_scalar(mean1[:], mean1[:], 1.0 / B,
                                            None, mybir.AluOpType.mult)
                    _moments(nc, small, big, a1t, 32, None, msq1[:],
                             f"t{t}")
                    nc.vector.tensor_scalar(msq1[:], msq1[:], 1.0 / B, EPS,
                                            mybir.AluOpType.mult,
                                            mybir.AluOpType.add)
                    nc.vector.tensor_tensor(ve1[:], mean1[:], mean1[:],
                                            mybir.AluOpType.mult)
                    nc.vector.tensor_tensor(ve1[:], msq1[:], ve1[:],
                                            mybir.AluOpType.subtract)
                    s1, c1 = _bn_scale_bias(nc, small, mean1, ve1,
                                            ga1_s[:, t:t + 1],
                                            bea1_s[:, t:t + 1],
                                            magic, 32, 1, "1")
                    a1a = auxbuf.tile([32, NC_FREE, BC], F32, tag="a1a",
                                      name=f"a1a_{t}", bufs=2)
                    nc.vector.tensor_scalar(a1a[:], a1t[:], s1[:], c1[:],
                                            mybir.AluOpType.mult,
                                            mybir.AluOpType.add)
                    nc.sync.dma_start(a1o[t * 32:(t + 1) * 32, :],
                                      a1a[:].rearrange("p a b -> p (a b)"))
    nc.compile()
    return nc


def prep_core_inputs(x, W0, b0, g0, be0, Wa0, ba0, ga0, bea0,
                     W1, b1, g1, be1, Wa1, ba1, ga1, bea1, core):
    f32 = np.float32
    sl0 = slice(core * M0L, (core + 1) * M0L)
    sl1 = slice(core * M1L, (core + 1) * M1L)
    out = {}
    out["xT"] = np.ascontiguousarray(x.T, dtype=f32)
    out["w0t"] = np.ascontiguousarray(
        W0[sl0].transpose(1, 0, 2).reshape(D_IN, M0L * H), dtype=f32)

    comb = np.zeros((NG, 128, 96), f32)
    wa0c = Wa0[sl0].reshape(NG, TG, H, 2)
    gi = np.arange(NG)[:, None, None, None]
    par = (np.arange(NG) % 2 * 48)[:, None, None, None]  # parity col offset
    mi = np.arange(TG)[None, :, None, None]
    hi = np.arange(H)[None, None, :, None]
    ki = np.arange(2)[None, None, None, :]
    comb[gi, mi * 8 + hi, par + mi * 2 + ki] = wa0c
    w1c = W1[sl1].reshape(NG, 2, FANIN * H, H)
    ri = np.arange(2)[None, :, None, None]
    ci = np.arange(FANIN * H)[None, None, :, None]
    h2i = np.arange(H)[None, None, None, :]
    comb[gi, ci + 64 * ri, par + 32 + ri * 8 + h2i] = w1c
    out["comb"] = comb

    comb1 = np.zeros((128, NT1, 32), f32)
    wa1c = Wa1[sl1].reshape(NT1, 16, H, 2)
    # comb1[r*8+h2, t, r*2+k] = wa1c[t, r, h2, k]
    for t in range(NT1):
        for r in range(16):
            comb1[r * 8:(r + 1) * 8, t, r * 2:(r + 1) * 2] = wa1c[t, r]
    out["comb1"] = comb1

    def col128(a):  # [M0L, H] -> [128, NG]
        return np.ascontiguousarray(
            a[sl0].reshape(NG, TG, H).transpose(1, 2, 0).reshape(128, NG), dtype=f32)
    out["b0t"], out["g0t"], out["be0t"] = col128(b0), col128(g0), col128(be0)

    def colaux(a0_, a1_):  # -> [96, NP]; rows q*48+[0:48] per group parity
        A0 = a0_[sl0].reshape(NG, TG, 2).transpose(1, 2, 0).reshape(32, NG)
        A1 = a1_[sl1].reshape(NG, 2, H).transpose(1, 2, 0).reshape(16, NG)
        A = np.concatenate([A0, A1], axis=0)  # [48, NG]
        return np.ascontiguousarray(
            A.reshape(48, NP, 2).transpose(2, 0, 1).reshape(96, NP), dtype=f32)
    out["bauxt"] = colaux(ba0, b1)
    out["gauxt"] = colaux(ga0, g1)
    out["beauxt"] = colaux(bea0, be1)

    def col32(a):  # [M1L, 2] -> [32, NT1]
        return np.ascontiguousarray(
            a[sl1].reshape(NT1, 16, 2).transpose(1, 2, 0).reshape(32, NT1), dtype=f32)
    out["b1at"], out["ga1t"], out["bea1t"] = col32(ba1), col32(ga1), col32(bea1)
    return out


def _install_ntff_hook():
    """The agent image's antenv lacks axon_hooks; shim it so trace=True can
    capture NTFF profiles (exec_time_ns) through the axon redirect."""
    import types
    try:
        from antenv.axon_hooks import get_axon_ntff_profile_hook  # noqa: F401
        return  # already present
    except ImportError:
        pass
    try:
        from trn_agent_boot.trn_boot import _ntff_profile_via_ctypes
        import antenv
        mod = types.ModuleType("antenv.axon_hooks")
        mod._hook = _ntff_profile_via_ctypes("/opt/axon/libaxon_pjrt.so")
        mod.set_axon_ntff_profile_hook = lambda h: setattr(mod, "_hook", h)
        mod.get_axon_ntff_profile_hook = lambda: mod._hook
        sys.modules["antenv.axon_hooks"] = mod
        antenv.axon_hooks = mod
    except Exception as e:  # degrade: no trace, run still works
        print(f"ntff hook shim failed ({e}); tracing disabled", file=sys.stderr)


def kernel(**inputs):
    global LAST_EXEC_TIME_NS
    from concourse.bass_utils import run_bass_kernel_spmd

    inputs = {k: np.asarray(v, dtype=np.float32) for k, v in inputs.items()}
    in_maps = [prep_core_inputs(core=c, **inputs) for c in range(N_CORES)]
    nc = build()
    trace = os.environ.get("KERNEL_TRACE", "0") == "1"
    if trace:
        _install_ntff_hook()
    res = run_bass_kernel_spmd(nc, in_maps, core_ids=list(range(N_CORES)),
                               trace=trace)
    LAST_EXEC_TIME_NS = res.exec_time_ns
    outs = res.results

    def gather(key, feat):
        per = [outs[c][key] for c in range(N_CORES)]
        per = [a.T.reshape(B, -1, feat) for a in per]
        return np.concatenate(per, axis=1)

    aux0 = gather("a0o", 2)
    aux1 = gather("a1o", 2)
    t0 = gather("t0o", H)
    t1 = gather("t1o", H)
    return (aux0, aux1, t0, t1)


# revision 19
# speedup vs baseline: 1.5063x; 1.5063x over previous
"""Trainium2 Bass kernel for nn_AOP_NN_49168785604711 (gnn_message_passing).

Two-layer ontology MLP:
  layer0: 8192 terms, each Linear(64->8)+tanh+BN(8), aux Linear(8->2)+tanh+BN(2)
  layer1: 1024 roots, concat 8 children (64) -> Linear(64->8)+tanh+BN, aux same.

Sharding: term axis split across 8 cores (expert parallel, no collectives).
Per core: 1024 MIE terms / 128 roots. x replicated.

Device layout: (term,feature) on SBUF partitions, batch on the free axis.
BatchNorm batch stats are then native free-axis reductions (bn_stats).
Outputs are written in [term*feat, B] layout; the host transposes (untimed).
"""

import os
import sys
import numpy as np

sys.path.insert(0, "/opt/trn_rl_repo")

from concourse import bass, bacc, mybir, tile  # noqa: E402

F32 = mybir.dt.float32
F32R = mybir.dt.float32r
BF16 = mybir.dt.bfloat16
I32 = mybir.dt.int32

B = 2048
D_IN = 64
H = 8
M0 = 8192
M1 = 1024
FANIN = 8
N_CORES = 8
EPS = 1e-5

M0L = M0 // N_CORES          # 1024 terms per core
M1L = M1 // N_CORES          # 128 roots per core
TG = 16                      # terms per group (16*8 = 128 partitions)
NG = M0L // TG               # 64 groups per core
NP = NG // 2                 # 32 group-pairs
NT1 = M1L * H // 128         # 8 t1 tiles of [128, B]
NC_FREE = 4                  # B split into 4 chunks of 512 for matmul/bn_stats
BC = B // NC_FREE            # 512

MAGIC = 0x5F3759DF

# matmul operand dtype: float32r streams 1 row/cycle (vs 4 for float32)
MM_DT = F32R if os.environ.get("KERNEL_MM_F32R", "1") == "1" else F32
# tanh-output dtype: bf16 halves bn_stats + apply input traffic
ACT_DT = BF16 if os.environ.get("KERNEL_BF16_ACT", "1") == "1" else F32

LAST_EXEC_TIME_NS = None


NR_ITERS = int(os.environ.get("KERNEL_NR_ITERS", "1"))


def _bn_scale_bias(nc, pool, agg, gcols, becols, magic, p, n, tag):
    """Batched BN scalars: agg [p, n, 2] (mean,var) + gamma/beta [p, n]
    -> s, c [p, n].  rstd via quake seed + NR_ITERS Newton steps."""
    ve = pool.tile([p, n], F32, tag=f"bn_ve{tag}", name=f"ve_{tag}")
    mu = agg[:, :, 0:1].rearrange("p a b -> p (a b)")
    var = agg[:, :, 1:2].rearrange("p a b -> p (a b)")
    nc.vector.tensor_scalar(ve[:], var, EPS, None, mybir.AluOpType.add)
    y = pool.tile([p, n], F32, tag=f"nw_y{tag}", name=f"y_{tag}")
    t = pool.tile([p, n], F32, tag=f"nw_t{tag}", name=f"t_{tag}")
    sh = pool.tile([p, n], I32, tag=f"nw_s{tag}", name=f"sh_{tag}")
    nc.vector.tensor_scalar(sh[:], ve[:].bitcast(I32), 1, None,
                            mybir.AluOpType.logical_shift_right)
    nc.vector.tensor_tensor(y[:].bitcast(I32), magic[:p, :n], sh[:],
                            mybir.AluOpType.subtract)
    for _ in range(NR_ITERS):
        nc.vector.tensor_tensor(t[:], y[:], y[:], mybir.AluOpType.mult)
        nc.vector.tensor_tensor(t[:], t[:], ve[:], mybir.AluOpType.mult)
        nc.vector.tensor_scalar(t[:], t[:], -0.5, 1.5,
                                mybir.AluOpType.mult, mybir.AluOpType.add)
        nc.vector.tensor_tensor(y[:], y[:], t[:], mybir.AluOpType.mult)
    s = pool.tile([p, n], F32, tag=f"bn_s{tag}", name=f"s_{tag}")
    c = pool.tile([p, n], F32, tag=f"bn_c{tag}", name=f"c_{tag}")
    nc.vector.tensor_tensor(s[:], gcols, y[:], mybir.AluOpType.mult)
    nc.vector.tensor_tensor(c[:], mu, s[:], mybir.AluOpType.mult)
    nc.vector.tensor_tensor(c[:], becols, c[:], mybir.AluOpType.subtract)
    return s, c


def build(n_groups=NG):
    """Per-core Bass graph, software-pipelined: stage A (matmul+tanh+stats)
    of pair p+1 is emitted before stage B (scalars+apply+aux) of pair p."""
    nc = bacc.Bacc(target_bir_lowering=False, debug=False)
    n_pairs = n_groups // 2
    n_t1 = max(1, n_groups // 8)

    xT = nc.declare_dram_parameter("xT", [D_IN, B], MM_DT, isOutput=False)
    w0t = nc.declare_dram_parameter("w0t", [D_IN, n_groups * 128], MM_DT, isOutput=False)
    comb = nc.declare_dram_parameter("comb", [n_groups, 128, 96], MM_DT, isOutput=False)
    comb1 = nc.declare_dram_parameter("comb1", [128, n_t1, 32], MM_DT, isOutput=False)
    b0t = nc.declare_dram_parameter("b0t", [128, n_groups], F32, isOutput=False)
    g0t = nc.declare_dram_parameter("g0t", [128, n_groups], F32, isOutput=False)
    be0t = nc.declare_dram_parameter("be0t", [128, n_groups], F32, isOutput=False)
    bauxt = nc.declare_dram_parameter("bauxt", [96, n_pairs], F32, isOutput=False)
    gauxt = nc.declare_dram_parameter("gauxt", [96, n_pairs], F32, isOutput=False)
    beauxt = nc.declare_dram_parameter("beauxt", [96, n_pairs], F32, isOutput=False)
    b1at = nc.declare_dram_parameter("b1at", [32, n_t1], F32, isOutput=False)
    ga1t = nc.declare_dram_parameter("ga1t", [32, n_t1], F32, isOutput=False)
    bea1t = nc.declare_dram_parameter("bea1t", [32, n_t1], F32, isOutput=False)

    t0o = nc.declare_dram_parameter("t0o", [n_groups * 128, B], MM_DT, isOutput=True)
    a0o = nc.declare_dram_parameter("a0o", [n_groups * 32, B], MM_DT, isOutput=True)
    t1o = nc.declare_dram_parameter("t1o", [n_t1 * 128, B], MM_DT, isOutput=True)
    a1o = nc.declare_dram_parameter("a1o", [n_t1 * 32, B], F32, isOutput=True)

    with tile.TileContext(nc) as tc:
        with (
            tc.tile_pool(name="const", bufs=1) as cpool,
            tc.tile_pool(name="wload", bufs=4) as wpool,
            tc.tile_pool(name="big", bufs=4) as big,
            tc.tile_pool(name="auxbuf", bufs=3) as auxbuf,
            tc.tile_pool(name="t1pool", bufs=2) as t1pool,
            tc.tile_pool(name="small", bufs=6) as small,
            tc.tile_pool(name="pmain", bufs=2, space="PSUM") as pmain,
            tc.tile_pool(name="paux", bufs=4, space="PSUM") as paux,
        ):
            xt_s = cpool.tile([D_IN, B], MM_DT)
            nc.sync.dma_start(xt_s[:], xT[:, :])
            b0_s = cpool.tile([128, n_groups], F32)
            g0_s = cpool.tile([128, n_groups], F32)
            be0_s = cpool.tile([128, n_groups], F32)
            nc.sync.dma_start(b0_s[:], b0t[:, :])
            nc.sync.dma_start(g0_s[:], g0t[:, :])
            nc.sync.dma_start(be0_s[:], be0t[:, :])
            baux_s = cpool.tile([96, n_pairs], F32)
            gaux_s = cpool.tile([96, n_pairs], F32)
            beaux_s = cpool.tile([96, n_pairs], F32)
            nc.sync.dma_start(baux_s[:], bauxt[:, :])
            nc.sync.dma_start(gaux_s[:], gauxt[:, :])
            nc.sync.dma_start(beaux_s[:], beauxt[:, :])
            c1_s = cpool.tile([128, n_t1, 32], MM_DT)
            nc.sync.dma_start(c1_s[:], comb1[:, :, :])
            b1a_s = cpool.tile([32, n_t1], F32)
            ga1_s = cpool.tile([32, n_t1], F32)
            bea1_s = cpool.tile([32, n_t1], F32)
            nc.sync.dma_start(b1a_s[:], b1at[:, :])
            nc.sync.dma_start(ga1_s[:], ga1t[:, :])
            nc.sync.dma_start(bea1_s[:], bea1t[:, :])
            magic = cpool.tile([128, 8], I32)
            nc.vector.memset(magic[:], MAGIC)

            stateA = {}
            state4 = {}

            def stage_a(p):
                h0s, cbs = [], []
                agg2 = small.tile([128, 2, 2], F32, tag="agg2",
                                  name=f"agg2_{p}")
                for q in (0, 1):
                    g = 2 * p + q
                    w0g = wpool.tile([D_IN, 128], MM_DT, tag="w0g",
                                     name=f"w0g_{g}")
                    nc.sync.dma_start(w0g[:], w0t[:, g * 128:(g + 1) * 128])
                    cb = wpool.tile([128, 96], MM_DT, tag="cb", name=f"cb_{g}")
                    nc.sync.dma_start(cb[:], comb[g, :, :])
                    cbs.append(cb)
                    h0 = big.tile([128, NC_FREE, BC], ACT_DT, tag="h0",
                                  name=f"h0_{g}")
                    h0s.append(h0)
                    for half in (0, 1):
                        ps = pmain.tile([128, 2, BC], F32, tag="pmain",
                                        name=f"ps_{g}_{half}")
                        for j in (0, 1):
                            nc.tensor.matmul(
                                ps[:, j, :],
                                w0g[:],
                                xt_s[:, (2 * half + j) * BC:(2 * half + j + 1) * BC],
                                start=True, stop=True)
                        nc.scalar.activation(
                            h0[:, 2 * half:2 * half + 2, :], ps[:, :, :],
                            mybir.ActivationFunctionType.Tanh,
                            bias=b0_s[:, g:g + 1])
                    st6 = small.tile([128, NC_FREE, 6], F32, tag="st6",
                                     name=f"st6_{g}")
                    for j in range(NC_FREE):
                        nc.vector.bn_stats(st6[:, j, :], h0[:, j, :])
                    nc.vector.bn_aggr(agg2[:, q, :], st6[:])
                stateA[p] = (h0s, cbs, agg2)

            def stage_b(p):
                h0s, cbs, agg2 = stateA.pop(p)
                if p % 4 == 0:
                    state4["t1buf"] = t1pool.tile(
                        [128, NC_FREE, BC], MM_DT, tag="t1buf",
                        name=f"t1buf_{p}")
                    state4["agga4"] = small.tile(
                        [96, 4, 2], F32, tag="agga4", name=f"agga4_{p}")
                t1buf = state4["t1buf"]
                agga4 = state4["agga4"]
                s2, c2 = _bn_scale_bias(nc, small, agg2,
                                        g0_s[:, 2 * p:2 * p + 2],
                                        be0_s[:, 2 * p:2 * p + 2],
                                        magic, 128, 2, "m")
                psa = [paux.tile([96, BC], F32, tag="paux",
                                 name=f"psa_{p}_{i}") for i in range(NC_FREE)]
                for q in (0, 1):
                    g = 2 * p + q
                    t0t = big.tile([128, NC_FREE, BC], MM_DT, tag="t0t",
                                   name=f"t0t_{g}")
                    nc.gpsimd.tensor_scalar(t0t[:], h0s[q][:],
                                            s2[:, q:q + 1], c2[:, q:q + 1],
                                            mybir.AluOpType.mult,
                                            mybir.AluOpType.add)
                    nc.sync.dma_start(t0o[g * 128:(g + 1) * 128, :],
                                      t0t[:].rearrange("p a b -> p (a b)"))
                    for j in range(NC_FREE):
                        nc.tensor.matmul(
                            psa[j][:, :],
                            cbs[q][:],
                            t0t[:, j, :],
                            start=(q == 0), stop=(q == 1))
                auxt = auxbuf.tile([96, NC_FREE, BC], ACT_DT, tag="auxt",
                                   name=f"auxt_{p}", bufs=6)
                state4[f"auxt_{p}"] = auxt
                for j in range(NC_FREE):
                    nc.scalar.activation(auxt[:, j, :], psa[j][:, :],
                                         mybir.ActivationFunctionType.Tanh,
                                         bias=baux_s[:, p:p + 1])
                st6a = small.tile([96, NC_FREE, 6], F32, tag="st6a",
                                  name=f"st6a_{p}")
                for j in range(NC_FREE):
                    nc.vector.bn_stats(st6a[:, j, :], auxt[:, j, :])
                nc.vector.bn_aggr(agga4[:, p % 4, :], st6a[:])

                if p % 4 == 3:
                    p0 = p - 3
                    sa4, ca4 = _bn_scale_bias(nc, small, agga4,
                                              gaux_s[:, p0:p0 + 4],
                                              beaux_s[:, p0:p0 + 4],
                                              magic, 96, 4, "a")
                    for k in range(4):
                        pk = p0 + k
                        auxa = auxbuf.tile([96, NC_FREE, BC], MM_DT,
                                           tag="auxa", name=f"auxa_{pk}",
                                           bufs=2)
                        nc.gpsimd.tensor_scalar(
                            auxa[:], state4.pop(f"auxt_{pk}")[:],
                            sa4[:, k:k + 1], ca4[:, k:k + 1],
                            mybir.AluOpType.mult, mybir.AluOpType.add)
                        ge, go = 2 * pk, 2 * pk + 1
                        nc.sync.dma_start(
                            a0o[ge * 32:(ge + 1) * 32, :],
                            auxa[0:32].rearrange("p a b -> p (a b)"))
                        nc.sync.dma_start(
                            a0o[go * 32:(go + 1) * 32, :],
                            auxa[48:80].rearrange("p a b -> p (a b)"))
                        sl_e = (ge % 8) * 16
                        sl_o = (go % 8) * 16
                        nc.sync.dma_start(t1buf[sl_e:sl_e + 16, :, :],
                                          auxa[32:48, :, :])
                        nc.sync.dma_start(t1buf[sl_o:sl_o + 16, :, :],
                                          auxa[80:96, :, :])

                    t = p // 4
                    nc.sync.dma_start(t1o[t * 128:(t + 1) * 128, :],
                                      t1buf[:].rearrange("p a b -> p (a b)"))
                    a1t = auxbuf.tile([32, NC_FREE, BC], ACT_DT, tag="a1t",
                                      name=f"a1t_{t}", bufs=2)
                    for j in range(NC_FREE):
                        ps1 = paux.tile([32, BC], F32, tag="paux",
                                        name=f"ps1_{t}_{j}")
                        nc.tensor.matmul(ps1[:, :],
                                         c1_s[:, t, :],
                                         t1buf[:, j, :],
                                         start=True, stop=True)
                        nc.scalar.activation(a1t[:, j, :], ps1[:, :],
                                             mybir.ActivationFunctionType.Tanh,
                                             bias=b1a_s[:, t:t + 1])
                    st61 = small.tile([32, NC_FREE, 6], F32, tag="st61",
                                      name=f"st61_{t}")
                    for j in range(NC_FREE):
                        nc.vector.bn_stats(st61[:, j, :], a1t[:, j, :])
                    agg1 = small.tile([32, 1, 2], F32, tag="agg1",
                                      name=f"agg1_{t}")
                    nc.vector.bn_aggr(agg1[:, 0, :], st61[:])
                    s1, c1 = _bn_scale_bias(nc, small, agg1,
                                            ga1_s[:, t:t + 1],
                                            bea1_s[:, t:t + 1],
                                            magic, 32, 1, "1")
                    a1a = auxbuf.tile([32, NC_FREE, BC], F32, tag="a1a",
                                      name=f"a1a_{t}", bufs=2)
                    nc.vector.tensor_scalar(a1a[:], a1t[:], s1[:], c1[:],
                                            mybir.AluOpType.mult,
                                            mybir.AluOpType.add)
                    nc.sync.dma_start(a1o[t * 32:(t + 1) * 32, :],
                                      a1a[:].rearrange("p a b -> p (a b)"))

            for p in range(n_pairs):
                stage_a(p)
                if p >= 1:
                    stage_b(p - 1)
            stage_b(n_pairs - 1)
    nc.compile()
    return nc


def prep_core_inputs(x, W0, b0, g0, be0, Wa0, ba0, ga0, bea0,
                     W1, b1, g1, be1, Wa1, ba1, ga1, bea1, core):
    f32 = np.float32
    sl0 = slice(core * M0L, (core + 1) * M0L)
    sl1 = slice(core * M1L, (core + 1) * M1L)
    out = {}
    out["xT"] = np.ascontiguousarray(x.T, dtype=f32)
    out["w0t"] = np.ascontiguousarray(
        W0[sl0].transpose(1, 0, 2).reshape(D_IN, M0L * H), dtype=f32)

    comb = np.zeros((NG, 128, 96), f32)
    wa0c = Wa0[sl0].reshape(NG, TG, H, 2)
    gi = np.arange(NG)[:, None, None, None]
    par = (np.arange(NG) % 2 * 48)[:, None, None, None]  # parity col offset
    mi = np.arange(TG)[None, :, None, None]
    hi = np.arange(H)[None, None, :, None]
    ki = np.arange(2)[None, None, None, :]
    comb[gi, mi * 8 + hi, par + mi * 2 + ki] = wa0c
    w1c = W1[sl1].reshape(NG, 2, FANIN * H, H)
    ri = np.arange(2)[None, :, None, None]
    ci = np.arange(FANIN * H)[None, None, :, None]
    h2i = np.arange(H)[None, None, None, :]
    comb[gi, ci + 64 * ri, par + 32 + ri * 8 + h2i] = w1c
    out["comb"] = comb

    comb1 = np.zeros((128, NT1, 32), f32)
    wa1c = Wa1[sl1].reshape(NT1, 16, H, 2)
    # comb1[r*8+h2, t, r*2+k] = wa1c[t, r, h2, k]
    for t in range(NT1):
        for r in range(16):
            comb1[r * 8:(r + 1) * 8, t, r * 2:(r + 1) * 2] = wa1c[t, r]
    out["comb1"] = comb1

    def col128(a):  # [M0L, H] -> [128, NG]
        return np.ascontiguousarray(
            a[sl0].reshape(NG, TG, H).transpose(1, 2, 0).reshape(128, NG), dtype=f32)
    out["b0t"], out["g0t"], out["be0t"] = col128(b0), col128(g0), col128(be0)

    def colaux(a0_, a1_):  # -> [96, NP]; rows q*48+[0:48] per group parity
        A0 = a0_[sl0].reshape(NG, TG, 2).transpose(1, 2, 0).reshape(32, NG)
        A1 = a1_[sl1].reshape(NG, 2, H).transpose(1, 2, 0).reshape(16, NG)
        A = np.concatenate([A0, A1], axis=0)  # [48, NG]
        return np.ascontiguousarray(
            A.reshape(48, NP, 2).transpose(2, 0, 1).reshape(96, NP), dtype=f32)
    out["bauxt"] = colaux(ba0, b1)
    out["gauxt"] = colaux(ga0, g1)
    out["beauxt"] = colaux(bea0, be1)

    def col32(a):  # [M1L, 2] -> [32, NT1]
        return np.ascontiguousarray(
            a[sl1].reshape(NT1, 16, 2).transpose(1, 2, 0).reshape(32, NT1), dtype=f32)
    out["b1at"], out["ga1t"], out["bea1t"] = col32(ba1), col32(ga1), col32(bea1)
    return out


def _install_ntff_hook():
    """The agent image's antenv lacks axon_hooks; shim it so trace=True can
    capture NTFF profiles (exec_time_ns) through the axon redirect."""
    import types
    try:
        from antenv.axon_hooks import get_axon_ntff_profile_hook  # noqa: F401
        return  # already present
    except ImportError:
        pass
    try:
        from trn_agent_boot.trn_boot import _ntff_profile_via_ctypes
        import antenv
        mod = types.ModuleType("antenv.axon_hooks")
        mod._hook = _ntff_profile_via_ctypes("/opt/axon/libaxon_pjrt.so")
        mod.set_axon_ntff_profile_hook = lambda h: setattr(mod, "_hook", h)
        mod.get_axon_ntff_profile_hook = lambda: mod._hook
        sys.modules["antenv.axon_hooks"] = mod
        antenv.axon_hooks = mod
    except Exception as e:  # degrade: no trace, run still works
        print(f"ntff hook shim failed ({e}); tracing disabled", file=sys.stderr)


def kernel(**inputs):
    global LAST_EXEC_TIME_NS
    from concourse.bass_utils import run_bass_kernel_spmd

    inputs = {k: np.asarray(v, dtype=np.float32) for k, v in inputs.items()}
    in_maps = [prep_core_inputs(core=c, **inputs) for c in range(N_CORES)]
    nc = build()
    trace = os.environ.get("KERNEL_TRACE", "0") == "1"
    if trace:
        _install_ntff_hook()
    res = run_bass_kernel_spmd(nc, in_maps, core_ids=list(range(N_CORES)),
                               trace=trace)
    LAST_EXEC_TIME_NS = res.exec_time_ns
    outs = res.results

    def gather(key, feat):
        per = [outs[c][key] for c in range(N_CORES)]
        per = [a.T.reshape(B, -1, feat) for a in per]
        return np.concatenate(per, axis=1)

    aux0 = gather("a0o", 2)
    aux1 = gather("a1o", 2)
    t0 = gather("t0o", H)
    t1 = gather("t1o", H)
    return (aux0, aux1, t0, t1)
